# revision 20
# baseline (speedup 1.0000x reference)
"""Trainium2 Bass kernel for a pre-LN transformer block (B=2,S=2048,H=1024,NH=16,FFN=4096).

Sharding: 8 cores, 512 tokens/core (4 cores per batch element). K/V exchanged
within each batch group via fp8 AllGathers triggered early. All dense
projections (QKV, attn-proj, FFN) run as fp8e4 DoubleRow matmuls (two
128-deep K blocks per instruction -> half the PE stream cycles, which is what
matters on power-throttled silicon); weights are pre-scaled x32 into fp8's
normal range and the scales are undone exactly via ACT scale params / cheap
DVE multiplies. Attention scores/ctx stay plain fp8 (they hide under the
scalar-engine exp stream, the true floor of the kernel).

Self-contained: hardcodes shapes; builds the Bass program once and runs it via
run_bass_kernel_spmd on cores 0-7.
"""

import sys

for _p in ("/root/.axon_site/_ro/trn_rl_repo", "/opt/trn_rl_repo"):
    if _p not in sys.path:
        sys.path.append(_p)

import numpy as np
import ml_dtypes

# If BASS_TRACE is set but the axon NTFF hook module is missing, the trace
# path would crash on import; pre-register a no-op hook shim so tracing
# degrades gracefully instead.
try:
    import antenv.axon_hooks  # noqa: F401
except ImportError:
    import types as _types
    _m = _types.ModuleType("antenv.axon_hooks")
    _m._hook = None
    _m.get_axon_ntff_profile_hook = lambda: _m._hook
    _m.set_axon_ntff_profile_hook = lambda h: setattr(_m, "_hook", h)
    sys.modules["antenv.axon_hooks"] = _m

import bass_rust
import concourse.bass as bass
import concourse.mybir as mybir
import concourse.tile as tile
from concourse.bass_utils import run_bass_kernel_spmd

BF16 = mybir.dt.bfloat16
F32 = mybir.dt.float32
FP8 = mybir.dt.float8e4
DR = mybir.MatmulPerfMode.DoubleRow
AF = mybir.ActivationFunctionType
NPBF16 = np.dtype(ml_dtypes.bfloat16)
NPFP8 = np.dtype(mybir.dt.np(FP8))

B, S, H, NH, DH, FFN = 2, 2048, 1024, 16, 64, 4096
NC = 8                      # cores
T = 512                     # tokens per core
NT = T // 128               # token tiles per core (4)
GROUPS = [[0, 1, 2, 3], [4, 5, 6, 7]]
G = 4                       # cores per batch group
SKEYS = S                   # keys per batch (2048)
NKT = SKEYS // 128          # key tiles (16)
NHP = NH // 2               # head pairs (8)
EPS = 1e-3
VW = DH + 1                 # 65: V columns + ones column per head
WS = 32.0                   # fp8 weight pre-scale
FFN_FP8 = False             # wi/wo fp8-DR failed the 2e-2 gate (measured 2.4e-2)
# key tiles in half-A (first V AllGather) then half-B order
KT_HALF_A = [g * 4 + j for g in range(4) for j in (0, 1)]
KT_HALF_B = [g * 4 + 2 + j for g in range(4) for j in (0, 1)]

# ---------------------------------------------------------------------------
# Workaround: this walrus build rejects >1 inline sync-wait per instruction.
# After Tile scheduling, move excess waits onto single-wait NoOp carriers
# inserted immediately before the over-limit instruction (same engine, same
# block, so per-engine program order and wait semantics are preserved).
# ---------------------------------------------------------------------------
def _split_multiwait(nc, limit=1):
    n_new = 0
    for f in nc.m.functions:
        for blk in f.blocks:
            insts = blk.instructions
            out = []
            for ins in insts:
                si = getattr(ins, "sync_info", None)
                waits = list(si.on_wait) if si is not None else []
                if len(waits) > limit:
                    for i, w in enumerate(waits[:-limit]):
                        nop = mybir.InstNoOp(
                            name=f"{ins.name}_w{i}",
                            sync_info=mybir.SyncInfo(on_wait=[w], on_update=[]),
                            bass_nofuse=True,
                            engine=ins.engine,
                        )
                        out.append(nop)
                        n_new += 1
                    ins.sync_info = mybir.SyncInfo(
                        on_wait=waits[-limit:], on_update=list(si.on_update)
                    )
                out.append(ins)
            if len(out) != len(insts):
                blk.instructions = out
    return n_new


def _emit(tc, nc, io):
    """Emit the per-core program. io: dict of DRAM APs."""
    from contextlib import ExitStack

    x_d = io["x"]
    out_d = io["out"]

    s_outer = ExitStack()

    constp = s_outer.enter_context(tc.tile_pool(name="constp", bufs=1))
    dramp = s_outer.enter_context(tc.tile_pool(name="dramp", bufs=1, space="DRAM"))

    # tiny warmup AllGather: absorbs the ~30us first-collective setup cost
    # while QKV compute proceeds, so the real AGs run at steady-state speed
    cc_warm_in = dramp.tile([128, 4], FP8)
    cc_warm_out = dramp.tile([G * 128, 4], FP8)
    nc.gpsimd.collective_compute(
        "AllGather", mybir.AluOpType.bypass, replica_groups=GROUPS,
        ins=[cc_warm_in.opt()], outs=[cc_warm_out.opt()],
    )

    # constants / biases
    ident = constp.tile([128, 128], BF16)
    nc.sync.dma_start(ident[:], io["ident"][:])
    ones_row = constp.tile([1, 128], BF16)
    nc.sync.dma_start(ones_row[:], io["ones_row"][:])
    bq = constp.tile([128, 8], F32); nc.sync.dma_start(bq[:], io["bq"][:])
    bk = constp.tile([128, 8], F32); nc.sync.dma_start(bk[:], io["bk"][:])
    bi = constp.tile([128, 32], F32); nc.sync.dma_start(bi[:], io["bi"][:])
    bv_bf = constp.tile([1, H], BF16); nc.sync.dma_start(bv_bf[:], io["bv_bf"][:])
    bproj_bf = constp.tile([1, H], BF16); nc.sync.dma_start(bproj_bf[:], io["bproj_bf"][:])
    bo_bf = constp.tile([1, H], BF16); nc.sync.dma_start(bo_bf[:], io["bo_bf"][:])
    eps_t = constp.tile([128, 1], F32); nc.gpsimd.memset(eps_t[:], float(EPS))

    # collective buffers (fp8)
    cc_k_in_a = dramp.tile([512, T], FP8)
    cc_k_out_a = dramp.tile([G * 512, T], FP8)
    cc_m_in = dramp.tile([1536, T], FP8)      # [kb 512 | va 512 | vb 512] rows
    cc_m_out = dramp.tile([G * 1536, T], FP8)

    # persistent activations
    persp = s_outer.enter_context(tc.tile_pool(name="persp", bufs=1))
    x_all = persp.tile([128, NT * H], F32, name="x_all")
    qT_all = persp.tile([128, 8 * T], FP8, name="qT_all")
    ctxT_all = persp.tile([128, 8 * T], FP8, name="ctxT_all")
    wproj_sb = persp.tile([128, 8 * H], FP8, name="wproj_sb")

    # ---- weight loads: 4 consolidated DMAs on the ACT HW-DGE queue, emitted
    # before any ACT compute so the triggers fire in the first few us ----
    from contextlib import ExitStack as _ES
    sA2 = _ES()   # stays open until the last qT inside phase B
    wq_p = sA2.enter_context(tc.tile_pool(name="wq_p", bufs=3))
    h1Tp = sA2.enter_context(tc.tile_pool(name="h1Tp", bufs=1))
    h1T_all = h1Tp.tile([128, 8 * T], FP8)

    def load_w(dst_sb, src_d):
        nc.scalar.dma_start(
            dst_sb[:].rearrange("p (f h) -> p f h", h=H),
            src_d.rearrange("(f p) h -> p f h", p=128))

    wk_sb = wq_p.tile([128, 8 * H], FP8, tag="w3", name="wk_sb")
    wq_sb = wq_p.tile([128, 8 * H], FP8, tag="w3", name="wq_sb")
    wv_sb = wq_p.tile([128, 8 * H], FP8, tag="w3", name="wv_sb")
    load_w(wk_sb, io["wk"])
    load_w(wq_sb, io["wq"])
    load_w(wv_sb, io["wv"])
    load_w(wproj_sb, io["wproj"])

    def layer_norm_to(pool, h_out_slice, x_slice):
        """x_slice [128,H] f32 -> h_out_slice [128,H] standardized."""
        sq = pool.tile([128, H], F32, tag="ln_sq")
        nsum = pool.tile([128, 1], F32, tag="ln_nsum")
        s2 = pool.tile([128, 1], F32, tag="ln_s2")
        var = pool.tile([128, 1], F32, tag="ln_var")
        std = pool.tile([128, 1], F32, tag="ln_std")
        rs = pool.tile([128, 1], F32, tag="ln_rs")
        nmu = pool.tile([128, 1], F32, tag="ln_nmu")
        nmurs = pool.tile([128, 1], F32, tag="ln_nmurs")
        nc.vector.reduce_sum(nsum[:], x_slice, axis=mybir.AxisListType.X, negate=True)
        nc.vector.tensor_mul(sq[:], x_slice, x_slice)
        nc.vector.reduce_sum(s2[:], sq[:], axis=mybir.AxisListType.X)
        nc.vector.tensor_scalar_mul(nmu[:], nsum[:], 1.0 / H)      # -mean
        nc.vector.tensor_scalar_mul(s2[:], s2[:], 1.0 / H)         # E[x^2]
        nc.vector.tensor_mul(var[:], nmu[:], nmu[:])               # mean^2
        nc.vector.tensor_sub(var[:], s2[:], var[:])                # var
        nc.scalar.activation(std[:], var[:], AF.Sqrt, bias=eps_t[:])
        nc.vector.reciprocal(rs[:], std[:])
        nc.vector.tensor_mul(nmurs[:], nmu[:], rs[:])              # -mean*rs
        nc.scalar.activation(h_out_slice, x_slice, AF.Identity, bias=nmurs[:], scale=rs[:])

    def transpose_128(dst_slice, src_slice, tps):
        """PE-transpose src [128,128] bf16; the PSUM->SBUF copy may convert."""
        ps = tps.tile([128, 128], BF16, tag="tp")
        nc.tensor.transpose(ps[:], src_slice, ident[:])
        nc.vector.tensor_copy(dst_slice, ps[:])

    # =====================================================================
    # Phase A: load x, LN1, h1^T (bf16 transpose path, fp8 store);
    # k^T -> AG(Ka),AG(Kb); v -> AG(Va),AG(Vb). Weight DMAs ride the ACT
    # HW-DGE queue. qT head pairs 0-1 here; 2-7 inside the B schedule.
    # =====================================================================
    sA = ExitStack()
    lnp = sA.enter_context(tc.tile_pool(name="lnp", bufs=2))
    h1p = sA.enter_context(tc.tile_pool(name="h1p", bufs=1))
    tpsA = sA.enter_context(tc.tile_pool(name="tpsA", bufs=2, space="PSUM"))
    mmpsA = sA.enter_context(tc.tile_pool(name="mmpsA", bufs=2, space="PSUM"))
    stgA = sA.enter_context(tc.tile_pool(name="stgA", bufs=4))

    h1_all = h1p.tile([128, NT * H], BF16)

    for t in range(NT):
        nc.sync.dma_start(x_all[:, t * H:(t + 1) * H], x_d[t * 128:(t + 1) * 128, :])

    for t in range(NT):
        layer_norm_to(lnp, h1_all[:, t * H:(t + 1) * H], x_all[:, t * H:(t + 1) * H])
        for fb in range(8):
            transpose_128(
                h1T_all[:, fb * T + t * 128: fb * T + (t + 1) * 128],
                h1_all[:, t * H + fb * 128: t * H + (fb + 1) * 128],
                tpsA,
            )

    h1T_v = h1T_all[:].rearrange("p (f t) -> p f t", t=T)

    # k^T feature-major [128 feats, T], fp8 out; DoubleRow over fb pairs
    def emit_kt(ct):
        ps = mmpsA.tile([128, T], F32, tag="mm_qk")
        wv_ = wk_sb[:].rearrange("p (f h) -> p f h", h=H)
        for i in range(4):
            nc.tensor.matmul(
                ps[:],
                wv_[:, 2 * i:2 * i + 2, ct * 128:(ct + 1) * 128],
                h1T_v[:, 2 * i:2 * i + 2, :],
                start=(i == 0), stop=(i == 3), perf_mode=DR,
            )
        ktmp = stgA.tile([128, T], FP8, tag="ktmp")
        nc.scalar.activation(ktmp[:], ps[:], AF.Identity, bias=bk[:, ct:ct + 1],
                             scale=1.0 / WS)
        dst = cc_k_in_a if ct < 4 else cc_m_in
        nc.sync.dma_start(dst[(ct % 4) * 128:(ct % 4 + 1) * 128, :], ktmp[:])

    def emit_qt(ct, pool, tag):
        ps = pool.tile([128, T], F32, tag=tag)
        wv_ = wq_sb[:].rearrange("p (f h) -> p f h", h=H)
        for i in range(4):
            nc.tensor.matmul(
                ps[:],
                wv_[:, 2 * i:2 * i + 2, ct * 128:(ct + 1) * 128],
                h1T_v[:, 2 * i:2 * i + 2, :],
                start=(i == 0), stop=(i == 3), perf_mode=DR,
            )
        nc.scalar.activation(
            qT_all[:, ct * T:(ct + 1) * T], ps[:], AF.Identity,
            bias=bq[:, ct:ct + 1], scale=1.0 / WS)

    for ct in range(4):
        emit_kt(ct)
    nc.gpsimd.collective_compute(
        "AllGather", mybir.AluOpType.bypass, replica_groups=GROUPS,
        ins=[cc_k_in_a.opt()], outs=[cc_k_out_a.opt()],
    )
    for ct in range(4, 8):
        emit_kt(ct)
    emit_qt(0, mmpsA, "mm_qk")
    emit_qt(1, mmpsA, "mm_qk")


    def emit_v(t):
        wv_ = wv_sb[:].rearrange("p (f h) -> p f h", h=H)
        for cc in range(2):
            ps = mmpsA.tile([128, 512], F32, tag="mm_v")
            for i in range(4):
                nc.tensor.matmul(
                    ps[:],
                    h1T_v[:, 2 * i:2 * i + 2, t * 128:(t + 1) * 128],
                    wv_[:, 2 * i:2 * i + 2, cc * 512:(cc + 1) * 512],
                    start=(i == 0), stop=False, perf_mode=DR,
                )
            nc.tensor.matmul(ps[:], ones_row[:], bv_bf[:, cc * 512:(cc + 1) * 512],
                             start=False, stop=True, skip_group_check=True)
            vtmp = stgA.tile([128, 512], FP8, tag="vtmp")
            nc.vector.tensor_scalar_mul(vtmp[:], ps[:], 1.0 / WS)
            half = 0 if t < 2 else 1
            b = (t % 2) * 2 + cc
            r0 = 512 + half * 512 + b * 128
            nc.sync.dma_start(cc_m_in[r0:r0 + 128, :], vtmp[:])

    emit_v(0); emit_v(1); emit_v(2); emit_v(3)
    nc.gpsimd.collective_compute(
        "AllGather", mybir.AluOpType.bypass, replica_groups=GROUPS,
        ins=[cc_m_in.opt()], outs=[cc_m_out.opt()],
    )

    sA.close()


    # =====================================================================
    # Phase B: attention.
    #   scores^T per key-tile (row-packed head pairs, fp8 q/k), exp on ACT
    #   into a deep fp8 pb ring. ctx token-major: psum[tok,65] += pb_chunk^T
    #   @ [V|1] accumulated over key tiles; normalize = per-partition
    #   1/sumexp (tiny DVE ops, x32 fold for the fp8 ctx store); PE
    #   transposes restore the feat-major ctx^T layout proj expects.
    # =====================================================================
    sB = ExitStack()
    kpool = sB.enter_context(tc.tile_pool(name="kpool", bufs=4))
    spool = sB.enter_context(tc.tile_pool(name="spool", bufs=2, space="PSUM"))
    cpool = sB.enter_context(tc.tile_pool(name="cpool", bufs=2, space="PSUM"))
    tpsB = sB.enter_context(tc.tile_pool(name="tpsB", bufs=2, space="PSUM"))
    ppool = sB.enter_context(tc.tile_pool(name="ppool", bufs=68))
    rpool = sB.enter_context(tc.tile_pool(name="rpool", bufs=8))
    ctokp = sB.enter_context(tc.tile_pool(name="ctokp", bufs=4))
    vsb = sB.enter_context(tc.tile_pool(name="vsb_p", bufs=1)).tile([128, NKT * NH * VW], FP8, name="vsb")

    def load_kt(hp):
        kt_hp = kpool.tile([128, SKEYS], FP8, tag="kt_hp", name="kt_hp")
        for g in range(G):
            if hp < 4:
                src_ = cc_k_out_a[g * 512 + hp * 128: g * 512 + (hp + 1) * 128, :]
            else:
                src_ = cc_m_out[g * 1536 + (hp - 4) * 128: g * 1536 + (hp - 3) * 128, :]
            nc.sync.dma_start(kt_hp[:, g * T:(g + 1) * T], src_)
        return kt_hp

    # ones columns for all key tiles up front (DVE; no data deps)
    for kt in range(NKT):
        blk = vsb[:, kt * NH * VW:(kt + 1) * NH * VW]
        nc.vector.memset(blk.rearrange("p (h x) -> p h x", x=VW)[:, :, DH:VW], 1.0)

    def load_v_half(half_kts, half):
        # V from merged AG output straight into the interleaved vsb layout;
        # the [128,H] token-block of (g, half, j) lives as two [128,512]
        # row-blocks b = j*2 + cc at rows g*1536 + 512 + half*512 + b*128.
        for kt in half_kts:
            g, j = kt // 4, (kt % 4) % 2
            blk = vsb[:, kt * NH * VW:(kt + 1) * NH * VW]
            dst = blk.rearrange("p (h x) -> p h x", x=VW)[:, :, 0:DH]
            for cc in range(2):
                r0 = g * 1536 + 512 + half * 512 + (j * 2 + cc) * 128
                src_ = cc_m_out[r0:r0 + 128, :]
                nc.sync.dma_start(dst[:, cc * 8:(cc + 1) * 8, :],
                                  src_.rearrange("p (h d) -> p h d", d=DH))

    # DMA emission order == AllGather completion order (Ka, Va, Vb, Kb)
    kt_tiles = [None] * NHP
    for hp in range(4):
        kt_tiles[hp] = load_kt(hp)
    load_v_half(KT_HALF_A, 0)
    load_v_half(KT_HALF_B, 1)
    for hp in range(4, NHP):
        kt_tiles[hp] = load_kt(hp)

    KT_ORDER = KT_HALF_A + KT_HALF_B
    pb_tiles = {}   # (hp, kt) -> pb tile (live between exp and ctx)
    cps_tiles = {}  # hp -> (cps0, cps1), each [128, 4*VW] f32 (one psum bank)

    def emit_scores(hp):
        kt_hp = kt_tiles[hp]
        for kt in KT_ORDER:
            ps = spool.tile([128, 1024], F32, tag="ps", name="ps")
            nc.tensor.matmul(
                ps[:, 0:512],
                kt_hp[0:64, kt * 128:(kt + 1) * 128],
                qT_all[0:64, hp * T:(hp + 1) * T],
                start=True, stop=True, tile_position=(0, 0),
            )
            nc.tensor.matmul(
                ps[:, 512:1024],
                kt_hp[64:128, kt * 128:(kt + 1) * 128],
                qT_all[64:128, hp * T:(hp + 1) * T],
                start=True, stop=True, tile_position=(64, 0),
            )
            pb = ppool.tile([128, 1024], FP8, tag="pb", name="pb")
            nc.scalar.activation(pb[:], ps[:], AF.Exp)
            pb_tiles[(hp, kt)] = pb

    def emit_ctx(hp, kts, first, last):
        """token-major ctx: psum[tok 128, VW] per (head, token chunk)."""
        if first:
            cps_tiles[hp] = (
                cpool.tile([128, NT * VW], F32, tag="cps", name=f"cps0_{hp}"),
                cpool.tile([128, NT * VW], F32, tag="cps", name=f"cps1_{hp}"),
            )
        cps0, cps1 = cps_tiles[hp]
        for i, kt in enumerate(kts):
            pb = pb_tiles.pop((hp, kt))
            st = first and i == 0
            sp = last and i == len(kts) - 1
            for h, cps in enumerate((cps0, cps1)):
                head = hp * 2 + h
                vv = vsb[:, kt * NH * VW + head * VW: kt * NH * VW + (head + 1) * VW]
                for c in range(NT):
                    nc.tensor.matmul(
                        cps[:, c * VW:(c + 1) * VW],
                        pb[:, h * 512 + c * 128: h * 512 + (c + 1) * 128],
                        vv,
                        start=st, stop=sp,
                    )

    def emit_fin(hp):
        """normalize token-major ctx by WS/sumexp, transpose to ctxT_all."""
        cps0, cps1 = cps_tiles.pop(hp)
        for c in range(NT):
            ctok = ctokp.tile([128, 128], BF16, tag="ctok", name=f"ctok_{hp}_{c}")
            for h, cps in enumerate((cps0, cps1)):
                rs = rpool.tile([128, 1], F32, tag="rs")
                nc.vector.reciprocal(rs[:], cps[:, c * VW + DH: c * VW + DH + 1])
                nc.vector.tensor_scalar(
                    ctok[:, h * DH:(h + 1) * DH],
                    cps[:, c * VW: c * VW + DH], rs[:], WS,
                    mybir.AluOpType.mult, mybir.AluOpType.mult)
            transpose_128(
                ctxT_all[:, hp * T + c * 128: hp * T + (c + 1) * 128],
                ctok[:], tpsB)

    # schedule: scores two head pairs ahead of ctx; qT 2-7 interleaved just
    # in time; ctx split into AG-half chunks so the PE never long-stalls.
    emit_scores(0)
    emit_qt(2, spool, "ps")
    emit_scores(1)
    emit_qt(3, spool, "ps")
    emit_scores(2)
    emit_qt(4, spool, "ps")
    emit_scores(3)
    emit_qt(5, spool, "ps")
    for hp in range(NHP):
        emit_ctx(hp, KT_HALF_A, first=True, last=False)
        if hp + 4 < NHP:
            emit_scores(hp + 4)
        emit_ctx(hp, KT_HALF_B, first=False, last=True)
        if hp + 6 < NHP:
            emit_qt(hp + 6, spool, "ps")
        emit_fin(hp)

    sB.close()
    sA2.close()

    # =====================================================================
    # Phase C: proj (fp8 DoubleRow over head-pair blocks) + residual -> x2,
    # LN2 -> h2^T (fp8)
    # =====================================================================
    sCD = ExitStack()
    x2p = sCD.enter_context(tc.tile_pool(name="x2p", bufs=1))
    h2Tp = sCD.enter_context(tc.tile_pool(name="h2Tp", bufs=1))
    h3Tp = sCD.enter_context(tc.tile_pool(name="h3Tp", bufs=1))
    x2_all = x2p.tile([128, NT * H], F32, name="x2_all")
    FT = FP8 if FFN_FP8 else BF16
    h2T_all = h2Tp.tile([128, 8 * T], FT, name="h2T_all")
    h3T_all = h3Tp.tile([128, 32 * T], FT, name="h3T_all")

    sC = ExitStack()
    lnp2 = sC.enter_context(tc.tile_pool(name="lnp2", bufs=2))
    h2p = sC.enter_context(tc.tile_pool(name="h2p", bufs=1))
    tpsC = sC.enter_context(tc.tile_pool(name="tpsC", bufs=2, space="PSUM"))
    mmpsC = sC.enter_context(tc.tile_pool(name="mmpsC", bufs=2, space="PSUM"))
    stgC = sC.enter_context(tc.tile_pool(name="stgC", bufs=4))

    h2_all = h2p.tile([128, NT * H], BF16)
    ctxT_v = ctxT_all[:].rearrange("p (f t) -> p f t", t=T)
    wproj_v = wproj_sb[:].rearrange("p (f h) -> p f h", h=H)

    for t in range(NT):
        for cc in range(2):
            ps = mmpsC.tile([128, 512], F32, tag="pj")
            for i in range(4):
                nc.tensor.matmul(
                    ps[:],
                    ctxT_v[:, 2 * i:2 * i + 2, t * 128:(t + 1) * 128],
                    wproj_v[:, 2 * i:2 * i + 2, cc * 512:(cc + 1) * 512],
                    start=(i == 0), stop=False, perf_mode=DR,
                )
            nc.tensor.matmul(ps[:], ones_row[:], bproj_bf[:, cc * 512:(cc + 1) * 512],
                             start=False, stop=True, skip_group_check=True)
            sl = slice(t * H + cc * 512, t * H + (cc + 1) * 512)
            tmp = stgC.tile([128, 512], F32, tag="pjtmp")
            nc.vector.tensor_scalar_mul(tmp[:], ps[:], 1.0 / (WS * WS))
            nc.vector.tensor_add(x2_all[:, sl], tmp[:], x_all[:, sl])
        layer_norm_to(lnp2, h2_all[:, t * H:(t + 1) * H], x2_all[:, t * H:(t + 1) * H])
        for fb in range(8):
            transpose_128(
                h2T_all[:, fb * T + t * 128: fb * T + (t + 1) * 128],
                h2_all[:, t * H + fb * 128: t * H + (fb + 1) * 128],
                tpsC,
            )

    sC.close()

    # =====================================================================
    # Phase D+E fused: per g-tile: wi matmuls + gelu -> h3T[g], then wo
    # matmuls for output columns 0:512 accumulate into 4 persistent psums.
    # Second pass re-reads h3T for output columns 512:1024.
    # =====================================================================
    sD = ExitStack()
    wip = sD.enter_context(tc.tile_pool(name="wip", bufs=6))
    wop = sD.enter_context(tc.tile_pool(name="wop", bufs=6))
    mmpsD = sD.enter_context(tc.tile_pool(name="mmpsD", bufs=4, space="PSUM"))
    wops = sD.enter_context(tc.tile_pool(name="wops", bufs=1, space="PSUM"))
    outp = sD.enter_context(tc.tile_pool(name="outp", bufs=2))

    h2T_v = h2T_all[:].rearrange("p (f t) -> p f t", t=T)
    h3T_v = h3T_all[:].rearrange("p (g t) -> p g t", t=T)
    NG = FFN // 128  # 32
    ffn_sc = WS if FFN_FP8 else 1.0      # psE = WS * ffn_raw (h3 itself unscaled)
    gelu_sc = (1.0 / WS) if FFN_FP8 else 1.0

    def emit_wi(g):
        wi_g = wip.tile([128, 8, 128], FT, tag="wi_g", name="wi_g")
        src = io["wi"][g:g + 1, :, :, :].rearrange("o p f c -> (o p) f c")
        nc.sync.dma_start(wi_g[:], src)
        ps = mmpsD.tile([128, T], F32, tag="wi_ps", name="wi_ps")
        if FFN_FP8:
            for i in range(4):
                nc.tensor.matmul(
                    ps[:], wi_g[:, 2 * i:2 * i + 2, :], h2T_v[:, 2 * i:2 * i + 2, :],
                    start=(i == 0), stop=(i == 3), perf_mode=DR,
                )
        else:
            for fb in range(8):
                nc.tensor.matmul(
                    ps[:], wi_g[:, fb, :], h2T_all[:, fb * T:(fb + 1) * T],
                    start=(fb == 0), stop=(fb == 7),
                )
        nc.scalar.activation(h3T_all[:, g * T:(g + 1) * T], ps[:],
                             AF.Gelu_apprx_tanh, bias=bi[:, g:g + 1], scale=gelu_sc)

    def emit_wo(j, cols, psE, first):
        # one DoubleRow pair of g-blocks (or one g in bf16 mode)
        if FFN_FP8:
            wo_g = wop.tile([128, 2, 512], FT, tag="wo_g", name="wo_g")
            nc.sync.dma_start(
                wo_g[:],
                io["wo"][(2 * j) * 128:(2 * j + 2) * 128, cols].rearrange(
                    "(two p) h -> p two h", two=2))
            for t in range(NT):
                nc.tensor.matmul(
                    psE[t][:],
                    h3T_v[:, 2 * j:2 * j + 2, t * 128:(t + 1) * 128],
                    wo_g[:],
                    start=first, stop=False, perf_mode=DR,
                )
        else:
            wo_g = wop.tile([128, 512], FT, tag="wo_g", name="wo_g")
            nc.sync.dma_start(wo_g[:], io["wo"][j * 128:(j + 1) * 128, cols])
            for t in range(NT):
                nc.tensor.matmul(
                    psE[t][:],
                    h3T_all[:, j * T + t * 128: j * T + (t + 1) * 128],
                    wo_g[:],
                    start=first, stop=False,
                )

    def emit_out(psE, cols, c0):
        for t in range(NT):
            nc.tensor.matmul(psE[t][:], ones_row[:], bo_bf[:, cols], start=False,
                             stop=True, skip_group_check=True)
            ot = outp.tile([128, 512], F32, tag="ot", name="ot")
            tmp = outp.tile([128, 512], F32, tag="ottmp", name="ottmp")
            nc.vector.tensor_scalar_mul(tmp[:], psE[t][:], 1.0 / ffn_sc)
            nc.vector.tensor_add(ot[:], tmp[:], x2_all[:, t * H + c0: t * H + c0 + 512])
            nc.sync.dma_start(out_d[t * 128:(t + 1) * 128, c0:c0 + 512], ot[:])

    WO_STEP = 2 if FFN_FP8 else 1
    psE = [wops.tile([128, 512], F32, tag=f"wo_ps{t}", name=f"wo_ps{t}") for t in range(NT)]
    for g in range(NG):
        emit_wi(g)
        if g % WO_STEP == WO_STEP - 1:
            j = g // WO_STEP
            emit_wo(j, slice(0, 512), psE, first=(j == 0))
    emit_out(psE, slice(0, 512), 0)

    psE2 = [wops.tile([128, 512], F32, tag=f"wo_ps{t}", name=f"wo2_ps{t}") for t in range(NT)]
    for j in range(NG // WO_STEP):
        emit_wo(j, slice(512, 1024), psE2, first=(j == 0))
    emit_out(psE2, slice(512, 1024), 512)

    sD.close()
    sCD.close()
    s_outer.close()


def _build_program():
    nc = bass.Bass("TRN2", target_bir_lowering=False, debug=False, num_devices=NC)
    WT = FP8
    FT = FP8 if FFN_FP8 else BF16
    io = {}
    io["x"] = nc.dram_tensor("x", [T, H], F32, kind="ExternalInput").ap()
    io["wq"] = nc.dram_tensor("wq", [H, H], WT, kind="ExternalInput").ap()
    io["wk"] = nc.dram_tensor("wk", [H, H], WT, kind="ExternalInput").ap()
    io["wv"] = nc.dram_tensor("wv", [H, H], WT, kind="ExternalInput").ap()
    io["wproj"] = nc.dram_tensor("wproj", [H, H], WT, kind="ExternalInput").ap()
    io["wi"] = nc.dram_tensor("wi", [FFN // 128, 128, 8, 128], FT, kind="ExternalInput").ap()
    io["wo"] = nc.dram_tensor("wo", [FFN, H], FT, kind="ExternalInput").ap()
    io["bq"] = nc.dram_tensor("bq", [128, 8], F32, kind="ExternalInput").ap()
    io["bk"] = nc.dram_tensor("bk", [128, 8], F32, kind="ExternalInput").ap()
    io["bi"] = nc.dram_tensor("bi", [128, 32], F32, kind="ExternalInput").ap()
    io["bv_bf"] = nc.dram_tensor("bv_bf", [1, H], BF16, kind="ExternalInput").ap()
    io["bproj_bf"] = nc.dram_tensor("bproj_bf", [1, H], BF16, kind="ExternalInput").ap()
    io["bo_bf"] = nc.dram_tensor("bo_bf", [1, H], BF16, kind="ExternalInput").ap()
    io["ident"] = nc.dram_tensor("ident", [128, 128], BF16, kind="ExternalInput").ap()
    io["ones_row"] = nc.dram_tensor("ones_row", [1, 128], BF16, kind="ExternalInput").ap()
    io["out"] = nc.dram_tensor("out", [T, H], F32, kind="ExternalOutput").ap()
    with tile.TileContext(nc) as tc:
        _emit(tc, nc, io)
    _split_multiwait(nc)
    return nc


_PROGRAM = None
LAST_RESULTS = None


def kernel(x, ln1_scale, ln1_bias, qkv_w, qkv_b, proj_w, proj_b,
           ln2_scale, ln2_bias, wi_w, wi_b, wo_w, wo_b):
    global _PROGRAM, LAST_RESULTS
    x = np.asarray(x, np.float32)
    ln1_scale = np.asarray(ln1_scale, np.float32); ln1_bias = np.asarray(ln1_bias, np.float32)
    qkv_w = np.asarray(qkv_w, np.float32); qkv_b = np.asarray(qkv_b, np.float32)
    proj_w = np.asarray(proj_w, np.float32); proj_b = np.asarray(proj_b, np.float32)
    ln2_scale = np.asarray(ln2_scale, np.float32); ln2_bias = np.asarray(ln2_bias, np.float32)
    wi_w = np.asarray(wi_w, np.float32); wi_b = np.asarray(wi_b, np.float32)
    wo_w = np.asarray(wo_w, np.float32); wo_b = np.asarray(wo_b, np.float32)

    # fold LN affine params into the next matmul's weights/biases
    qkv_w_eff = ln1_scale[:, None] * qkv_w
    qkv_b_eff = qkv_b + ln1_bias @ qkv_w
    w3 = qkv_w_eff.reshape(H, NH, 3, DH)
    b3 = qkv_b_eff.reshape(NH, 3, DH)
    scale = 1.0 / np.sqrt(np.float32(DH))
    wq = w3[:, :, 0, :].reshape(H, H)          # no 1/sqrt(d): folded into bq/ACT
    wk = w3[:, :, 1, :].reshape(H, H)
    wv = w3[:, :, 2, :].reshape(H, H)
    bq_v = b3[:, 0, :].reshape(H)
    bk_v = b3[:, 1, :].reshape(H)
    bv_v = b3[:, 2, :].reshape(H)
    wi_eff = ln2_scale[:, None] * wi_w
    bi_v = wi_b + ln2_bias @ wi_w

    FT = NPFP8 if FFN_FP8 else NPBF16
    ffs = WS if FFN_FP8 else 1.0
    common = {
        # q path: weights x32 (no sqrt-d), ACT applies 1/(32*sqrt(d)) & bias/sqrt(d)
        "wq": (wq * (WS / np.sqrt(np.float32(DH)))).astype(NPFP8),
        "wk": (wk * WS).astype(NPFP8),
        "wv": (wv * WS).astype(NPFP8),
        "wproj": (proj_w * WS).astype(NPFP8),
        "wi": np.ascontiguousarray(
            (wi_eff * ffs).astype(FT).reshape(8, 128, 32, 128).transpose(2, 1, 0, 3)),
        "wo": (wo_w * ffs).astype(FT),
        "bq": np.ascontiguousarray((bq_v * scale).reshape(8, 128).T.astype(np.float32)),
        "bk": np.ascontiguousarray(bk_v.reshape(8, 128).T.astype(np.float32)),
        "bi": np.ascontiguousarray(bi_v.reshape(32, 128).T.astype(np.float32)),
        "bv_bf": (bv_v * WS).reshape(1, H).astype(NPBF16),
        "bproj_bf": (proj_b * WS * WS).reshape(1, H).astype(NPBF16),
        "bo_bf": (wo_b * ffs).reshape(1, H).astype(NPBF16),
        "ident": np.eye(128, dtype=NPBF16),
        "ones_row": np.ones((1, 128), NPBF16),
    }
    x_flat = x.reshape(B * S, H)
    in_maps = []
    for c in range(NC):
        m = dict(common)
        m["x"] = np.ascontiguousarray(x_flat[c * T:(c + 1) * T, :])
        in_maps.append(m)

    if _PROGRAM is None:
        _PROGRAM = _build_program()
    r = run_bass_kernel_spmd(_PROGRAM, in_maps, list(range(NC)))
    LAST_RESULTS = r
    out = np.concatenate([r.results[c]["out"] for c in range(NC)], axis=0)
    return out.reshape(B, S, H).astype(np.float32)
